# revision 20
# baseline (speedup 1.0000x reference)
"""GCN (3-layer, PyG-style) forward on 8 Trainium2 NeuronCores — fp8 v3.

Math restructuring (aggregation commutes with the weight matmul; pooling is
linear):
  agg1 = Anorm @ x                        # [N,2]  tiny -> host, f64
  h1   = relu(agg1 @ W1 + b1)             # [N,1024] exact f32 on HOST
  msg  = h1[row_e]                        # gathered per edge slot, fp8, DMA'd
  g2   = Anorm @ h1                       # fp8 DoubleRow scatter matmuls
  h2   = relu(g2 @ W2 + b2)               # fp8 DoubleRow dense matmuls
  pg3  = T.T @ h2  (T[src,g]=sum norm)    # bf16, accumulated in PSUM
  out  = ((pg3 @ W3 + cnt*b3)/max(cnt,1)) @ Wlin + blin   # host, f64

Per 128-edge tile the device consumes 1152 fp8 bytes per edge slot (1024 msg
features + 128 scatter-matrix row), streamed as one contiguous per-window DMA.
Power-of-two scale factors keep every fp8 tensor in range (lossless to fold).
Nodes are assigned to the 8*98 (core,window) bins by a balanced greedy (LPT on
in-degree) so every window needs the same number of 128-edge tiles.  g2 is
transposed with the XBAR DMA-transpose; pg3 stays resident in PSUM across all
windows.  The edge phase of window w+1 is emitted before the h2 phase of
window w so the PE never idles on the g2 copy/transpose.
"""

import numpy as np

N_NODES = 100000
N_EDGES = 400000
G = 128
FIN = 2
H = 1024
N_CORES = 8
P = 128
NW = 98                       # windows per core (98*128 = 12544 >= 12500)
NSLOT = NW * P                # node slots per core
NBIN = N_CORES * NW
EW = H + P                    # fp8 bytes per edge slot (msg + S row)
T_CH = 7                      # T windows per DMA chunk (98 = 14*7)
ALPHA_S = 4.0                 # power-of-2 scale on S (norm values)

LAST_RESULT = None


def _pow2(v):
    return float(2.0 ** np.round(np.log2(v)))


def _host_prep(x, edge_index, batch, W1, b1):
    """All O(E) index work in numpy; returns per-core device arrays."""
    import heapq
    import ml_dtypes

    f8 = ml_dtypes.float8_e4m3
    x = np.asarray(x, dtype=np.float32)
    ei = np.asarray(edge_index).astype(np.int64)
    batch = np.asarray(batch).astype(np.int64)
    n = N_NODES

    loops = np.arange(n, dtype=np.int64)
    row = np.concatenate([ei[0], loops])
    col = np.concatenate([ei[1], loops])

    deg = np.bincount(col, minlength=n).astype(np.float64)
    dis = np.where(deg > 0, 1.0 / np.sqrt(np.maximum(deg, 1.0)), 0.0)
    norm = dis[row] * dis[col]

    # layer-1 aggregation (FIN=2) on host, f64
    agg1 = np.empty((n, FIN), dtype=np.float32)
    for f in range(FIN):
        agg1[:, f] = np.bincount(
            col, weights=norm * x[row, f].astype(np.float64), minlength=n
        ).astype(np.float32)
    norm = norm.astype(np.float32)

    # exact h1 per node, then fp8 with a power-of-2 scale
    h1 = np.maximum(agg1 @ W1 + b1, 0.0)
    msg_rms = float(np.sqrt((h1[:4096] ** 2).mean()))
    alpha_m = _pow2(2.0 / msg_rms)
    h1q = (alpha_m * h1).astype(f8)

    # ---- balanced node -> (core, window, pos) assignment (LPT greedy) ----
    w_node = np.bincount(col, minlength=n)          # in-degree incl self-loop
    order = np.argsort(-w_node, kind="stable")
    bin_of = np.empty(n, dtype=np.int64)
    pos_of = np.empty(n, dtype=np.int64)
    counts = np.zeros(NBIN, dtype=np.int64)
    loads = np.zeros(NBIN, dtype=np.int64)
    heap = [(0, b) for b in range(NBIN)]
    heapq.heapify(heap)
    wl = w_node[order]
    for i in range(n):
        node = order[i]
        load, b = heapq.heappop(heap)
        bin_of[node] = b
        pos_of[node] = counts[b]
        counts[b] += 1
        loads[b] = load + wl[i]
        if counts[b] < P:
            heapq.heappush(heap, (loads[b], b))
    brank = np.argsort(-loads, kind="stable")
    core_of_bin = np.empty(NBIN, dtype=np.int64)
    win_of_bin = np.empty(NBIN, dtype=np.int64)
    core_of_bin[brank] = np.arange(NBIN) % N_CORES
    win_of_bin[brank] = np.arange(NBIN) // N_CORES

    node_core = core_of_bin[bin_of]
    node_win = win_of_bin[bin_of]
    node_pos = pos_of

    # ---- edge partition by destination bin ----
    ec = node_core[col]
    ew = node_win[col]
    ekey = ec * NW + ew
    eorder = np.argsort(ekey, kind="stable")
    row_s, col_s, norm_s = row[eorder], col[eorder], norm[eorder]
    ekey_s = ekey[eorder]

    cw_counts = np.bincount(ekey_s, minlength=NBIN).reshape(N_CORES, NW)
    tiles_per_cw = (cw_counts + P - 1) // P
    T_w = tiles_per_cw.max(axis=0)
    base_tile = np.concatenate([[0], np.cumsum(T_w)])
    TT = int(base_tile[-1])

    cw_starts = np.concatenate([[0], np.cumsum(cw_counts.reshape(-1))])
    idx_in_cw = np.arange(len(col_s)) - cw_starts[ekey_s]
    tile_in_w = idx_in_cw // P
    pos = idx_in_cw % P
    tile_global = base_tile[ekey_s % NW] + tile_in_w

    # per-window payload: [nt*1024 msg features | nt*128 S rows], all fp8.
    # msg pairs are 1024 apart and S pairs 128 apart -> contiguous DoubleRow
    # access patterns for the scatter matmuls.
    c = ec_s = ec[eorder]
    Amsg = np.zeros((N_CORES, P, TT, H), dtype=f8)
    Asca = np.zeros((N_CORES, P, TT, P), dtype=f8)
    Amsg[c, pos, tile_global] = h1q[row_s]
    Asca[c, pos, tile_global, node_pos[col_s]] = (ALPHA_S * norm_s).astype(f8)
    E = np.empty((N_CORES, P, TT * EW), dtype=f8)
    for w in range(NW):
        t0, nt = int(base_tile[w]), int(T_w[w])
        woff = t0 * EW
        mend = woff + nt * H
        E[:, :, woff:mend] = Amsg[:, :, t0 : t0 + nt].reshape(N_CORES, P, nt * H)
        E[:, :, mend : mend + nt * P] = Asca[:, :, t0 : t0 + nt].reshape(
            N_CORES, P, nt * P
        )
    del Amsg, Asca

    # ---- L3 pooling matrix T: rows indexed by source-node slot ----
    gcol = batch[col]
    rowslot = node_core[row] * NSLOT + node_win[row] * P + node_pos[row]
    Tmat = np.bincount(
        rowslot * G + gcol, weights=norm.astype(np.float64),
        minlength=N_CORES * NSLOT * G,
    ).astype(np.float32).reshape(N_CORES, NW, P, G)

    cnt = np.bincount(batch, minlength=G).astype(np.float32)
    return E, Tmat, cnt, T_w, TT, alpha_m


def _build_device_program(TT, T_w, evac_c, h2_c):
    import concourse.mybir as mybir
    import concourse.tile as tile
    from concourse import bacc

    f32 = mybir.dt.float32
    bf16 = mybir.dt.bfloat16
    f8 = mybir.dt.float8e4
    nc = bacc.Bacc(None, target_bir_lowering=False, debug=False)
    DR = mybir.MatmulPerfMode.DoubleRow

    E_d = nc.dram_tensor("E", [P, TT * EW], f8, kind="ExternalInput")
    T_d = nc.dram_tensor("T", [NW // T_CH, P, T_CH * G], bf16, kind="ExternalInput")
    W2_d = nc.dram_tensor("W2", [P, 8 * H], f8, kind="ExternalInput")
    b2_d = nc.dram_tensor("b2", [1, H], bf16, kind="ExternalInput")
    out_d = nc.dram_tensor("pg3", [G, H], f32, kind="ExternalOutput")

    Relu = mybir.ActivationFunctionType.Relu
    mult = mybir.AluOpType.mult
    vmax = mybir.AluOpType.max

    with tile.TileContext(nc) as tc:
        with (
            tc.tile_pool(name="const", bufs=1) as cst,
            tc.tile_pool(name="sE", bufs=3) as sE,
            tc.tile_pool(name="sT", bufs=2) as sT,
            tc.tile_pool(name="sg2s", bufs=3) as sg2s,
            tc.tile_pool(name="sg2T", bufs=3) as sg2T,
            tc.tile_pool(name="sg28", bufs=4) as sg28,
            tc.tile_pool(name="sh2", bufs=2) as sh2,
            tc.tile_pool(name="g2p", bufs=2, space="PSUM") as g2p,
            tc.tile_pool(name="hp", bufs=1, space="PSUM") as hp,
            tc.tile_pool(name="pgp", bufs=1, space="PSUM") as pgp,
        ):
            W2s = cst.tile([P, 8, H], f8, tag="W2s")
            nc.sync.dma_start(W2s[:], W2_d[:].rearrange("p (j f) -> p j f", j=8))
            b2s = cst.tile([1, H], bf16, tag="b2s")
            nc.sync.dma_start(b2s[:], b2_d[:])
            ones1 = cst.tile([1, P], bf16, tag="ones1")
            nc.vector.memset(ones1[:], 1.0)

            pg3 = pgp.tile([G, H], f32, tag="pg3")

            state = {"Ttc": None}
            pend = []  # (g2T8, Ttc, w)

            def edge_phase(w):
                nt = int(T_w[w])
                base = int(np.sum(T_w[:w]))
                if w % T_CH == 0:
                    state["Ttc"] = sT.tile(
                        [P, T_CH * G], bf16, tag="Ttc", name="Ttc"
                    )
                    nc.sync.dma_start(state["Ttc"][:], T_d[w // T_CH])
                if w % 2 == 0:
                    nt2 = int(T_w[w + 1]) if w + 1 < NW else 0
                    ln = (nt + nt2) * EW
                    Ec = sE.tile([P, 12 * EW], f8, tag="Ec", name="Ec")
                    nc.sync.dma_start(
                        Ec[:, :ln], E_d[:, base * EW : base * EW + ln]
                    )
                    state["Ec"] = Ec
                    state["Eoff"] = 0
                else:
                    Ec = state["Ec"]
                Eo = state["Eoff"]
                state["Eoff"] = Eo + nt * EW
                g2 = g2p.tile([P, H], f32, tag="g2")
                npair = nt // 2
                has_single = nt % 2 == 1
                soff = Eo + nt * H
                for pr in range(npair):
                    lhs = Ec[:, soff + 2 * pr * P : soff + (2 * pr + 2) * P]
                    lhs = lhs.rearrange("p (i m) -> p i m", i=2)
                    rhs2 = Ec[:, Eo + 2 * pr * H : Eo + (2 * pr + 2) * H]
                    rhs2 = rhs2.rearrange("p (i f) -> p i f", i=2)
                    for half in range(2):
                        lo = half * 512
                        nc.tensor.matmul(
                            g2[:, lo : lo + 512], lhs, rhs2[:, :, lo : lo + 512],
                            start=(pr == 0),
                            stop=(pr == npair - 1 and not has_single),
                            perf_mode=DR,
                        )
                if has_single:
                    t = nt - 1
                    lhs = Ec[:, soff + t * P : soff + (t + 1) * P]
                    for half in range(2):
                        lo = half * 512
                        nc.tensor.matmul(
                            g2[:, lo : lo + 512], lhs,
                            Ec[:, Eo + t * H + lo : Eo + t * H + lo + 512],
                            start=(npair == 0), stop=True,
                        )
                # evacuate (scaled) + transpose + cast for the h2 phase
                g2s = sg2s.tile([P, H], bf16, tag="g2s")
                nc.vector.tensor_scalar_mul(g2s[:, :512], g2[:, :512], evac_c)
                nc.scalar.activation(
                    g2s[:, 512:], g2[:, 512:],
                    mybir.ActivationFunctionType.Copy, scale=evac_c,
                )
                g2T = sg2T.tile([P, 8, P], bf16, tag="g2T")
                nc.scalar.dma_start_transpose(g2T[:], g2s[:])
                g28 = sg28.tile([P, 8, P], f8, tag="g28")
                nc.vector.tensor_copy(g28[:, :4], g2T[:, :4])
                nc.scalar.activation(
                    g28[:, 4:], g2T[:, 4:], mybir.ActivationFunctionType.Copy
                )
                pend.append((g28, state["Ttc"], w))

            def h2_pool_phase():
                g28, Ttc, w = pend.pop(0)
                toff = (w % T_CH) * G
                h2p = hp.tile([P, H], f32, tag="h2p")
                h2s = sh2.tile([P, H], bf16, tag="h2s")
                for half in range(2):
                    lo = half * 512
                    for i in range(4):
                        nc.tensor.matmul(
                            h2p[:, lo : lo + 512],
                            g28[:, 2 * i : 2 * i + 2, :],
                            W2s[:, 2 * i : 2 * i + 2, lo : lo + 512],
                            start=(i == 0), stop=False,
                            perf_mode=DR,
                        )
                    nc.tensor.matmul(
                        h2p[:, lo : lo + 512], ones1[:1, :], b2s[:1, lo : lo + 512],
                        start=False, stop=True,
                    )
                    if half == 0:
                        nc.vector.tensor_scalar(
                            h2s[:, :512], h2p[:, :512], h2_c, 0.0, mult, vmax
                        )
                    else:
                        nc.scalar.activation(
                            h2s[:, 512:], h2p[:, 512:], Relu, scale=h2_c
                        )
                for half in range(2):
                    lo = half * 512
                    nc.tensor.matmul(
                        pg3[:, lo : lo + 512],
                        Ttc[:, toff : toff + G],
                        h2s[:, lo : lo + 512],
                        start=(w == 0), stop=(w == NW - 1),
                    )

            for w in range(NW):
                if w >= 3:
                    h2_pool_phase()
                edge_phase(w)
            h2_pool_phase()
            h2_pool_phase()
            h2_pool_phase()

            pg3s = cst.tile([G, H], f32, tag="pg3s")
            nc.vector.tensor_copy(pg3s[:, :512], pg3[:, :512])
            nc.scalar.activation(
                pg3s[:, 512:], pg3[:, 512:], mybir.ActivationFunctionType.Copy
            )
            nc.sync.dma_start(out_d[:], pg3s[:])

    nc.finalize()
    return nc


def kernel(x, W1, b1, W2, b2, W3, b3, Wlin, blin, edge_index, batch, num_graphs):
    global LAST_RESULT
    import ml_dtypes
    from concourse.bass_utils import run_bass_kernel_spmd

    bf = ml_dtypes.bfloat16
    f8 = ml_dtypes.float8_e4m3
    x = np.asarray(x, dtype=np.float32)
    W1 = np.asarray(W1, dtype=np.float32)
    b1 = np.asarray(b1, dtype=np.float32)
    W2 = np.asarray(W2, dtype=np.float32)
    b2 = np.asarray(b2, dtype=np.float32)
    W3 = np.asarray(W3, dtype=np.float32)
    b3 = np.asarray(b3, dtype=np.float32)
    Wlin = np.asarray(Wlin, dtype=np.float32)
    blin = np.asarray(blin, dtype=np.float32)

    E, Tmat, cnt, T_w, TT, alpha_m = _host_prep(x, edge_index, batch, W1, b1)

    msg_rms_unscaled = 2.0 / alpha_m        # by construction of alpha_m
    gamma = _pow2(2.0 / (msg_rms_unscaled * 0.45))
    beta = _pow2(0.5 / float(np.sqrt((W2 ** 2).mean())))
    evac_c = gamma / (alpha_m * ALPHA_S)
    h2_c = 1.0 / (gamma * beta)

    nc = _build_device_program(TT, T_w, evac_c, h2_c)

    W2r = np.ascontiguousarray(
        (beta * W2).reshape(8, P, H).transpose(1, 0, 2).reshape(P, 8 * H)
    ).astype(f8)
    b2r = (gamma * beta * b2).reshape(1, H).astype(bf)

    in_maps = []
    for c in range(N_CORES):
        Ec = np.ascontiguousarray(E[c].reshape(P, TT * EW))
        Tc = np.ascontiguousarray(
            Tmat[c].reshape(NW // T_CH, T_CH, P, G).transpose(0, 2, 1, 3)
            .reshape(NW // T_CH, P, T_CH * G)
        ).astype(bf)
        in_maps.append({"E": Ec, "T": Tc, "W2": W2r, "b2": b2r})

    res = run_bass_kernel_spmd(nc, in_maps, core_ids=list(range(N_CORES)))
    LAST_RESULT = res
    pg3 = np.zeros((G, H), dtype=np.float64)
    for r in res.results:
        pg3 += r["pg3"].astype(np.float64)
    pg3 = pg3.astype(np.float32)

    pooled = (pg3 @ W3 + cnt[:, None] * b3[None, :]) / np.maximum(cnt, 1.0)[:, None]
    out = pooled @ Wlin + blin[None, :]
    return out.astype(np.float32)


# revision 22
# speedup vs baseline: 1.0554x; 1.0554x over previous
"""GCN (3-layer, PyG-style) forward on 8 Trainium2 NeuronCores — fp8 v3.

Math restructuring (aggregation commutes with the weight matmul; pooling is
linear):
  agg1 = Anorm @ x                        # [N,2]  tiny -> host, f64
  h1   = relu(agg1 @ W1 + b1)             # [N,1024] exact f32 on HOST
  msg  = h1[row_e]                        # gathered per edge slot, fp8, DMA'd
  g2   = Anorm @ h1                       # fp8 DoubleRow scatter matmuls
  h2   = relu(g2 @ W2 + b2)               # fp8 DoubleRow dense matmuls
  pg3  = T.T @ h2  (T[src,g]=sum norm)    # bf16, accumulated in PSUM
  out  = ((pg3 @ W3 + cnt*b3)/max(cnt,1)) @ Wlin + blin   # host, f64

Per 128-edge tile the device consumes 1152 fp8 bytes per edge slot (1024 msg
features + 128 scatter-matrix row), streamed as one contiguous per-window DMA.
Power-of-two scale factors keep every fp8 tensor in range (lossless to fold).
Nodes are assigned to the 8*98 (core,window) bins by a balanced greedy (LPT on
in-degree) so every window needs the same number of 128-edge tiles.  g2 is
transposed with the XBAR DMA-transpose; pg3 stays resident in PSUM across all
windows.  The edge phase of window w+1 is emitted before the h2 phase of
window w so the PE never idles on the g2 copy/transpose.
"""

import numpy as np

N_NODES = 100000
N_EDGES = 400000
G = 128
FIN = 2
H = 1024
N_CORES = 8
P = 128
NW = 98                       # windows per core (98*128 = 12544 >= 12500)
NSLOT = NW * P                # node slots per core
NBIN = N_CORES * NW
EW = H + P                    # fp8 bytes per edge slot (msg + S row)
T_CH = 7                      # T windows per DMA chunk (98 = 14*7)
ALPHA_S = 4.0                 # power-of-2 scale on S (norm values)

LAST_RESULT = None


def _pow2(v):
    return float(2.0 ** np.round(np.log2(v)))


def _host_prep(x, edge_index, batch, W1, b1):
    """All O(E) index work in numpy; returns per-core device arrays."""
    import heapq
    import ml_dtypes

    f8 = ml_dtypes.float8_e4m3
    x = np.asarray(x, dtype=np.float32)
    ei = np.asarray(edge_index).astype(np.int64)
    batch = np.asarray(batch).astype(np.int64)
    n = N_NODES

    loops = np.arange(n, dtype=np.int64)
    row = np.concatenate([ei[0], loops])
    col = np.concatenate([ei[1], loops])

    deg = np.bincount(col, minlength=n).astype(np.float64)
    dis = np.where(deg > 0, 1.0 / np.sqrt(np.maximum(deg, 1.0)), 0.0)
    norm = dis[row] * dis[col]

    # layer-1 aggregation (FIN=2) on host, f64
    agg1 = np.empty((n, FIN), dtype=np.float32)
    for f in range(FIN):
        agg1[:, f] = np.bincount(
            col, weights=norm * x[row, f].astype(np.float64), minlength=n
        ).astype(np.float32)
    norm = norm.astype(np.float32)

    # exact h1 per node, then fp8 with a power-of-2 scale
    h1 = np.maximum(agg1 @ W1 + b1, 0.0)
    msg_rms = float(np.sqrt((h1[:4096] ** 2).mean()))
    alpha_m = _pow2(2.0 / msg_rms)
    h1q = (alpha_m * h1).astype(f8)

    # ---- balanced node -> (core, window, pos) assignment (LPT greedy) ----
    w_node = np.bincount(col, minlength=n)          # in-degree incl self-loop
    order = np.argsort(-w_node, kind="stable")
    bin_of = np.empty(n, dtype=np.int64)
    pos_of = np.empty(n, dtype=np.int64)
    counts = np.zeros(NBIN, dtype=np.int64)
    loads = np.zeros(NBIN, dtype=np.int64)
    heap = [(0, b) for b in range(NBIN)]
    heapq.heapify(heap)
    wl = w_node[order]
    for i in range(n):
        node = order[i]
        load, b = heapq.heappop(heap)
        bin_of[node] = b
        pos_of[node] = counts[b]
        counts[b] += 1
        loads[b] = load + wl[i]
        if counts[b] < P:
            heapq.heappush(heap, (loads[b], b))
    brank = np.argsort(-loads, kind="stable")
    core_of_bin = np.empty(NBIN, dtype=np.int64)
    win_of_bin = np.empty(NBIN, dtype=np.int64)
    core_of_bin[brank] = np.arange(NBIN) % N_CORES
    win_of_bin[brank] = np.arange(NBIN) // N_CORES

    node_core = core_of_bin[bin_of]
    node_win = win_of_bin[bin_of]
    node_pos = pos_of

    # ---- edge partition by destination bin ----
    ec = node_core[col]
    ew = node_win[col]
    ekey = ec * NW + ew
    eorder = np.argsort(ekey, kind="stable")
    row_s, col_s, norm_s = row[eorder], col[eorder], norm[eorder]
    ekey_s = ekey[eorder]

    cw_counts = np.bincount(ekey_s, minlength=NBIN).reshape(N_CORES, NW)
    tiles_per_cw = (cw_counts + P - 1) // P
    T_w = tiles_per_cw.max(axis=0)
    base_tile = np.concatenate([[0], np.cumsum(T_w)])
    TT = int(base_tile[-1])

    cw_starts = np.concatenate([[0], np.cumsum(cw_counts.reshape(-1))])
    idx_in_cw = np.arange(len(col_s)) - cw_starts[ekey_s]
    tile_in_w = idx_in_cw // P
    pos = idx_in_cw % P
    tile_global = base_tile[ekey_s % NW] + tile_in_w

    # per-window payload: [nt*1024 msg features | nt*128 S rows], all fp8.
    # msg pairs are 1024 apart and S pairs 128 apart -> contiguous DoubleRow
    # access patterns for the scatter matmuls.
    c = ec_s = ec[eorder]
    Amsg = np.zeros((N_CORES, P, TT, H), dtype=f8)
    Asca = np.zeros((N_CORES, P, TT, P), dtype=f8)
    Amsg[c, pos, tile_global] = h1q[row_s]
    Asca[c, pos, tile_global, node_pos[col_s]] = (ALPHA_S * norm_s).astype(f8)
    E = np.empty((N_CORES, P, TT * EW), dtype=f8)
    for w in range(NW):
        t0, nt = int(base_tile[w]), int(T_w[w])
        woff = t0 * EW
        mend = woff + nt * H
        E[:, :, woff:mend] = Amsg[:, :, t0 : t0 + nt].reshape(N_CORES, P, nt * H)
        E[:, :, mend : mend + nt * P] = Asca[:, :, t0 : t0 + nt].reshape(
            N_CORES, P, nt * P
        )
    del Amsg, Asca

    # ---- L3 pooling matrix T: rows indexed by source-node slot ----
    gcol = batch[col]
    rowslot = node_core[row] * NSLOT + node_win[row] * P + node_pos[row]
    Tmat = np.bincount(
        rowslot * G + gcol, weights=norm.astype(np.float64),
        minlength=N_CORES * NSLOT * G,
    ).astype(np.float32).reshape(N_CORES, NW, P, G)

    cnt = np.bincount(batch, minlength=G).astype(np.float32)
    return E, Tmat, cnt, T_w, TT, alpha_m


def _build_device_program(TT, T_w, evac_c, h2_c):
    import concourse.mybir as mybir
    import concourse.tile as tile
    from concourse import bacc

    f32 = mybir.dt.float32
    bf16 = mybir.dt.bfloat16
    f8 = mybir.dt.float8e4
    nc = bacc.Bacc(None, target_bir_lowering=False, debug=False)
    DR = mybir.MatmulPerfMode.DoubleRow

    E_d = nc.dram_tensor("E", [P, TT * EW], f8, kind="ExternalInput")
    T_d = nc.dram_tensor("T", [NW // T_CH, P, T_CH * G], bf16, kind="ExternalInput")
    W2_d = nc.dram_tensor("W2", [P, 8 * H], f8, kind="ExternalInput")
    b2_d = nc.dram_tensor("b2", [1, H], bf16, kind="ExternalInput")
    out_d = nc.dram_tensor("pg3", [G, H], f32, kind="ExternalOutput")

    Relu = mybir.ActivationFunctionType.Relu
    mult = mybir.AluOpType.mult
    vmax = mybir.AluOpType.max

    with tile.TileContext(nc) as tc:
        with (
            tc.tile_pool(name="const", bufs=1) as cst,
            tc.tile_pool(name="sE", bufs=3) as sE,
            tc.tile_pool(name="sT", bufs=2) as sT,
            tc.tile_pool(name="sg2s", bufs=3) as sg2s,
            tc.tile_pool(name="sg2T", bufs=3) as sg2T,
            tc.tile_pool(name="sg28", bufs=4) as sg28,
            tc.tile_pool(name="sh2", bufs=2) as sh2,
            tc.tile_pool(name="g2p", bufs=2, space="PSUM") as g2p,
            tc.tile_pool(name="hp", bufs=1, space="PSUM") as hp,
            tc.tile_pool(name="pgp", bufs=1, space="PSUM") as pgp,
        ):
            W2s = cst.tile([P, 8, H], f8, tag="W2s")
            nc.sync.dma_start(W2s[:], W2_d[:].rearrange("p (j f) -> p j f", j=8))
            b2s = cst.tile([1, H], bf16, tag="b2s")
            nc.sync.dma_start(b2s[:], b2_d[:])
            ones1 = cst.tile([1, P], bf16, tag="ones1")
            nc.vector.memset(ones1[:], 1.0)

            pg3 = pgp.tile([G, H], f32, tag="pg3")

            state = {"Ttc": None, "pairs": {}}
            pend = []  # (g2T8, Ttc, w)
            base_tile = np.concatenate([[0], np.cumsum(T_w)]).astype(int)

            def load_pair(k):
                w0 = 2 * k
                if w0 >= NW:
                    return
                ln = int(T_w[w0]) * EW
                if w0 + 1 < NW:
                    ln += int(T_w[w0 + 1]) * EW
                Ec = sE.tile([P, 12 * EW], f8, tag="Ec", name="Ec")
                off = int(base_tile[w0]) * EW
                nc.sync.dma_start(Ec[:, :ln], E_d[:, off : off + ln])
                state["pairs"][k] = Ec

            def edge_phase(w):
                nt = int(T_w[w])
                if w % T_CH == 0:
                    state["Ttc"] = sT.tile(
                        [P, T_CH * G], bf16, tag="Ttc", name="Ttc"
                    )
                    nc.sync.dma_start(state["Ttc"][:], T_d[w // T_CH])
                if w % 2 == 0:
                    load_pair(w // 2 + 2)
                    Ec = state["pairs"].pop(w // 2)
                    state["Ec"] = Ec
                    state["Eoff"] = 0
                else:
                    Ec = state["Ec"]
                Eo = state["Eoff"]
                state["Eoff"] = Eo + nt * EW
                g2 = g2p.tile([P, H], f32, tag="g2")
                npair = nt // 2
                has_single = nt % 2 == 1
                soff = Eo + nt * H
                for pr in range(npair):
                    lhs = Ec[:, soff + 2 * pr * P : soff + (2 * pr + 2) * P]
                    lhs = lhs.rearrange("p (i m) -> p i m", i=2)
                    rhs2 = Ec[:, Eo + 2 * pr * H : Eo + (2 * pr + 2) * H]
                    rhs2 = rhs2.rearrange("p (i f) -> p i f", i=2)
                    for half in range(2):
                        lo = half * 512
                        nc.tensor.matmul(
                            g2[:, lo : lo + 512], lhs, rhs2[:, :, lo : lo + 512],
                            start=(pr == 0),
                            stop=(pr == npair - 1 and not has_single),
                            perf_mode=DR,
                        )
                if has_single:
                    t = nt - 1
                    lhs = Ec[:, soff + t * P : soff + (t + 1) * P]
                    for half in range(2):
                        lo = half * 512
                        nc.tensor.matmul(
                            g2[:, lo : lo + 512], lhs,
                            Ec[:, Eo + t * H + lo : Eo + t * H + lo + 512],
                            start=(npair == 0), stop=True,
                        )
                # evacuate (scaled) + transpose + cast for the h2 phase
                g2s = sg2s.tile([P, H], bf16, tag="g2s")
                nc.vector.tensor_scalar_mul(g2s[:, :512], g2[:, :512], evac_c)
                nc.scalar.activation(
                    g2s[:, 512:], g2[:, 512:],
                    mybir.ActivationFunctionType.Copy, scale=evac_c,
                )
                g2T = sg2T.tile([P, 8, P], bf16, tag="g2T")
                nc.scalar.dma_start_transpose(g2T[:], g2s[:])
                g28 = sg28.tile([P, 8, P], f8, tag="g28")
                nc.vector.tensor_copy(g28[:, :4], g2T[:, :4])
                nc.scalar.activation(
                    g28[:, 4:], g2T[:, 4:], mybir.ActivationFunctionType.Copy
                )
                pend.append((g28, state["Ttc"], w))

            def h2_pool_phase():
                g28, Ttc, w = pend.pop(0)
                toff = (w % T_CH) * G
                h2p = hp.tile([P, H], f32, tag="h2p")
                h2s = sh2.tile([P, H], bf16, tag="h2s")
                for half in range(2):
                    lo = half * 512
                    for i in range(4):
                        nc.tensor.matmul(
                            h2p[:, lo : lo + 512],
                            g28[:, 2 * i : 2 * i + 2, :],
                            W2s[:, 2 * i : 2 * i + 2, lo : lo + 512],
                            start=(i == 0), stop=False,
                            perf_mode=DR,
                        )
                    nc.tensor.matmul(
                        h2p[:, lo : lo + 512], ones1[:1, :], b2s[:1, lo : lo + 512],
                        start=False, stop=True,
                    )
                    if half == 0:
                        nc.vector.tensor_scalar(
                            h2s[:, :512], h2p[:, :512], h2_c, 0.0, mult, vmax
                        )
                    else:
                        nc.scalar.activation(
                            h2s[:, 512:], h2p[:, 512:], Relu, scale=h2_c
                        )
                for half in range(2):
                    lo = half * 512
                    nc.tensor.matmul(
                        pg3[:, lo : lo + 512],
                        Ttc[:, toff : toff + G],
                        h2s[:, lo : lo + 512],
                        start=(w == 0), stop=(w == NW - 1),
                    )

            load_pair(0)
            load_pair(1)
            for w in range(NW):
                if w >= 3:
                    h2_pool_phase()
                edge_phase(w)
            h2_pool_phase()
            h2_pool_phase()
            h2_pool_phase()

            pg3s = cst.tile([G, H], f32, tag="pg3s")
            nc.vector.tensor_copy(pg3s[:, :512], pg3[:, :512])
            nc.scalar.activation(
                pg3s[:, 512:], pg3[:, 512:], mybir.ActivationFunctionType.Copy
            )
            nc.sync.dma_start(out_d[:], pg3s[:])

    nc.finalize()
    return nc


def kernel(x, W1, b1, W2, b2, W3, b3, Wlin, blin, edge_index, batch, num_graphs):
    global LAST_RESULT
    import ml_dtypes
    from concourse.bass_utils import run_bass_kernel_spmd

    bf = ml_dtypes.bfloat16
    f8 = ml_dtypes.float8_e4m3
    x = np.asarray(x, dtype=np.float32)
    W1 = np.asarray(W1, dtype=np.float32)
    b1 = np.asarray(b1, dtype=np.float32)
    W2 = np.asarray(W2, dtype=np.float32)
    b2 = np.asarray(b2, dtype=np.float32)
    W3 = np.asarray(W3, dtype=np.float32)
    b3 = np.asarray(b3, dtype=np.float32)
    Wlin = np.asarray(Wlin, dtype=np.float32)
    blin = np.asarray(blin, dtype=np.float32)

    E, Tmat, cnt, T_w, TT, alpha_m = _host_prep(x, edge_index, batch, W1, b1)

    msg_rms_unscaled = 2.0 / alpha_m        # by construction of alpha_m
    gamma = _pow2(2.0 / (msg_rms_unscaled * 0.45))
    beta = _pow2(0.5 / float(np.sqrt((W2 ** 2).mean())))
    evac_c = gamma / (alpha_m * ALPHA_S)
    h2_c = 1.0 / (gamma * beta)

    nc = _build_device_program(TT, T_w, evac_c, h2_c)

    W2r = np.ascontiguousarray(
        (beta * W2).reshape(8, P, H).transpose(1, 0, 2).reshape(P, 8 * H)
    ).astype(f8)
    b2r = (gamma * beta * b2).reshape(1, H).astype(bf)

    in_maps = []
    for c in range(N_CORES):
        Ec = np.ascontiguousarray(E[c].reshape(P, TT * EW))
        Tc = np.ascontiguousarray(
            Tmat[c].reshape(NW // T_CH, T_CH, P, G).transpose(0, 2, 1, 3)
            .reshape(NW // T_CH, P, T_CH * G)
        ).astype(bf)
        in_maps.append({"E": Ec, "T": Tc, "W2": W2r, "b2": b2r})

    res = run_bass_kernel_spmd(nc, in_maps, core_ids=list(range(N_CORES)))
    LAST_RESULT = res
    pg3 = np.zeros((G, H), dtype=np.float64)
    for r in res.results:
        pg3 += r["pg3"].astype(np.float64)
    pg3 = pg3.astype(np.float32)

    pooled = (pg3 @ W3 + cnt[:, None] * b3[None, :]) / np.maximum(cnt, 1.0)[:, None]
    out = pooled @ Wlin + blin[None, :]
    return out.astype(np.float32)


# revision 23
# speedup vs baseline: 1.2531x; 1.1873x over previous
"""GCN (3-layer, PyG-style) forward on 8 Trainium2 NeuronCores — fp8 v3.

Math restructuring (aggregation commutes with the weight matmul; pooling is
linear):
  agg1 = Anorm @ x                        # [N,2]  tiny -> host, f64
  h1   = relu(agg1 @ W1 + b1)             # [N,1024] exact f32 on HOST
  msg  = h1[row_e]                        # gathered per edge slot, fp8, DMA'd
  g2   = Anorm @ h1                       # fp8 DoubleRow scatter matmuls
  h2   = relu(g2 @ W2 + b2)               # fp8 DoubleRow dense matmuls
  pg3  = T.T @ h2  (T[src,g]=sum norm)    # bf16, accumulated in PSUM
  out  = ((pg3 @ W3 + cnt*b3)/max(cnt,1)) @ Wlin + blin   # host, f64

Per 128-edge tile the device consumes 1152 fp8 bytes per edge slot (1024 msg
features + 128 scatter-matrix row), streamed as one contiguous per-window DMA.
Power-of-two scale factors keep every fp8 tensor in range (lossless to fold).
Nodes are assigned to the 8*98 (core,window) bins by a balanced greedy (LPT on
in-degree) so every window needs the same number of 128-edge tiles.  g2 is
transposed with the XBAR DMA-transpose; pg3 stays resident in PSUM across all
windows.  The edge phase of window w+1 is emitted before the h2 phase of
window w so the PE never idles on the g2 copy/transpose.
"""

import numpy as np

N_NODES = 100000
N_EDGES = 400000
G = 128
FIN = 2
H = 1024
N_CORES = 8
P = 128
NW = 98                       # windows per core (98*128 = 12544 >= 12500)
NSLOT = NW * P                # node slots per core
NBIN = N_CORES * NW
EW = H + P                    # fp8 bytes per edge slot (msg + S row)
T_CH = 7                      # T windows per DMA chunk (98 = 14*7)
ALPHA_S = 4.0                 # power-of-2 scale on S (norm values)

LAST_RESULT = None


def _pow2(v):
    return float(2.0 ** np.round(np.log2(v)))


def _host_prep(x, edge_index, batch, W1, b1):
    """All O(E) index work in numpy; returns per-core device arrays."""
    import heapq
    import ml_dtypes

    f8 = ml_dtypes.float8_e4m3
    x = np.asarray(x, dtype=np.float32)
    ei = np.asarray(edge_index).astype(np.int64)
    batch = np.asarray(batch).astype(np.int64)
    n = N_NODES

    loops = np.arange(n, dtype=np.int64)
    row = np.concatenate([ei[0], loops])
    col = np.concatenate([ei[1], loops])

    deg = np.bincount(col, minlength=n).astype(np.float64)
    dis = np.where(deg > 0, 1.0 / np.sqrt(np.maximum(deg, 1.0)), 0.0)
    norm = dis[row] * dis[col]

    # layer-1 aggregation (FIN=2) on host, f64
    agg1 = np.empty((n, FIN), dtype=np.float32)
    for f in range(FIN):
        agg1[:, f] = np.bincount(
            col, weights=norm * x[row, f].astype(np.float64), minlength=n
        ).astype(np.float32)
    norm = norm.astype(np.float32)

    # exact h1 per node, then fp8 with a power-of-2 scale
    h1 = np.maximum(agg1 @ W1 + b1, 0.0)
    msg_rms = float(np.sqrt((h1[:4096] ** 2).mean()))
    alpha_m = _pow2(2.0 / msg_rms)
    h1q = (alpha_m * h1).astype(f8)

    # ---- balanced node -> (core, window, pos) assignment (LPT greedy) ----
    w_node = np.bincount(col, minlength=n)          # in-degree incl self-loop
    order = np.argsort(-w_node, kind="stable")
    bin_of = np.empty(n, dtype=np.int64)
    pos_of = np.empty(n, dtype=np.int64)
    counts = np.zeros(NBIN, dtype=np.int64)
    loads = np.zeros(NBIN, dtype=np.int64)
    heap = [(0, b) for b in range(NBIN)]
    heapq.heapify(heap)
    wl = w_node[order]
    for i in range(n):
        node = order[i]
        load, b = heapq.heappop(heap)
        bin_of[node] = b
        pos_of[node] = counts[b]
        counts[b] += 1
        loads[b] = load + wl[i]
        if counts[b] < P:
            heapq.heappush(heap, (loads[b], b))
    brank = np.argsort(-loads, kind="stable")
    core_of_bin = np.empty(NBIN, dtype=np.int64)
    win_of_bin = np.empty(NBIN, dtype=np.int64)
    core_of_bin[brank] = np.arange(NBIN) % N_CORES
    win_of_bin[brank] = np.arange(NBIN) // N_CORES

    node_core = core_of_bin[bin_of]
    node_win = win_of_bin[bin_of]
    node_pos = pos_of

    # ---- edge partition by destination bin ----
    ec = node_core[col]
    ew = node_win[col]
    ekey = ec * NW + ew
    eorder = np.argsort(ekey, kind="stable")
    row_s, col_s, norm_s = row[eorder], col[eorder], norm[eorder]
    ekey_s = ekey[eorder]

    cw_counts = np.bincount(ekey_s, minlength=NBIN).reshape(N_CORES, NW)
    tiles_per_cw = (cw_counts + P - 1) // P
    T_w = tiles_per_cw.max(axis=0)
    base_tile = np.concatenate([[0], np.cumsum(T_w)])
    TT = int(base_tile[-1])

    cw_starts = np.concatenate([[0], np.cumsum(cw_counts.reshape(-1))])
    idx_in_cw = np.arange(len(col_s)) - cw_starts[ekey_s]
    tile_in_w = idx_in_cw // P
    pos = idx_in_cw % P
    tile_global = base_tile[ekey_s % NW] + tile_in_w

    # per-window payload: [nt*1024 msg features | nt*128 S rows], all fp8.
    # msg pairs are 1024 apart and S pairs 128 apart -> contiguous DoubleRow
    # access patterns for the scatter matmuls.
    c = ec_s = ec[eorder]
    Amsg = np.zeros((N_CORES, P, TT, H), dtype=f8)
    Asca = np.zeros((N_CORES, P, TT, P), dtype=f8)
    Amsg[c, pos, tile_global] = h1q[row_s]
    Asca[c, pos, tile_global, node_pos[col_s]] = (ALPHA_S * norm_s).astype(f8)
    E = np.empty((N_CORES, P, TT * EW), dtype=f8)
    for w in range(NW):
        t0, nt = int(base_tile[w]), int(T_w[w])
        woff = t0 * EW
        mend = woff + nt * H
        E[:, :, woff:mend] = Amsg[:, :, t0 : t0 + nt].reshape(N_CORES, P, nt * H)
        E[:, :, mend : mend + nt * P] = Asca[:, :, t0 : t0 + nt].reshape(
            N_CORES, P, nt * P
        )
    del Amsg, Asca

    # ---- L3 pooling matrix T: rows indexed by source-node slot ----
    gcol = batch[col]
    rowslot = node_core[row] * NSLOT + node_win[row] * P + node_pos[row]
    Tmat = np.bincount(
        rowslot * G + gcol, weights=norm.astype(np.float64),
        minlength=N_CORES * NSLOT * G,
    ).astype(np.float32).reshape(N_CORES, NW, P, G)

    cnt = np.bincount(batch, minlength=G).astype(np.float32)
    return E, Tmat, cnt, T_w, TT, alpha_m


def _build_device_program(TT, T_w, evac_c, h2_c):
    import concourse.mybir as mybir
    import concourse.tile as tile
    from concourse import bacc

    f32 = mybir.dt.float32
    bf16 = mybir.dt.bfloat16
    f8 = mybir.dt.float8e4
    nc = bacc.Bacc(None, target_bir_lowering=False, debug=False)
    DR = mybir.MatmulPerfMode.DoubleRow

    E_d = nc.dram_tensor("E", [P, TT * EW], f8, kind="ExternalInput")
    T_d = nc.dram_tensor("T", [NW // T_CH, P, T_CH * G], bf16, kind="ExternalInput")
    W2_d = nc.dram_tensor("W2", [P, 8 * H], f8, kind="ExternalInput")
    b2_d = nc.dram_tensor("b2", [1, H], bf16, kind="ExternalInput")
    out_d = nc.dram_tensor("pg3", [G, H], f32, kind="ExternalOutput")

    Relu = mybir.ActivationFunctionType.Relu
    mult = mybir.AluOpType.mult
    vmax = mybir.AluOpType.max

    with tile.TileContext(nc) as tc:
        with (
            tc.tile_pool(name="const", bufs=1) as cst,
            tc.tile_pool(name="sE", bufs=3) as sE,
            tc.tile_pool(name="sT", bufs=2) as sT,
            tc.tile_pool(name="sg2s", bufs=3) as sg2s,
            tc.tile_pool(name="sg2T", bufs=3) as sg2T,
            tc.tile_pool(name="sg28", bufs=4) as sg28,
            tc.tile_pool(name="sh2", bufs=2) as sh2,
            tc.tile_pool(name="g2p", bufs=2, space="PSUM") as g2p,
            tc.tile_pool(name="hp", bufs=1, space="PSUM") as hp,
            tc.tile_pool(name="pgp", bufs=1, space="PSUM") as pgp,
        ):
            W2s = cst.tile([P, 8, H], f8, tag="W2s")
            nc.sync.dma_start(W2s[:], W2_d[:].rearrange("p (j f) -> p j f", j=8))
            b2s = cst.tile([1, H], bf16, tag="b2s")
            nc.sync.dma_start(b2s[:], b2_d[:])
            ones1 = cst.tile([1, P], bf16, tag="ones1")
            nc.vector.memset(ones1[:], 1.0)

            pg3 = pgp.tile([G, H], f32, tag="pg3")

            state = {"Ttc": None, "pairs": {}, "Tchunks": {}}
            pend = []  # (g28 pair tile, Ttc, w0)
            base_tile = np.concatenate([[0], np.cumsum(T_w)]).astype(int)

            def load_pair(k):
                w0 = 2 * k
                if w0 >= NW:
                    return
                ln = int(T_w[w0]) * EW
                if w0 + 1 < NW:
                    ln += int(T_w[w0 + 1]) * EW
                Ec = sE.tile([P, 12 * EW], f8, tag="Ec", name="Ec")
                off = int(base_tile[w0]) * EW
                nc.sync.dma_start(Ec[:, :ln], E_d[:, off : off + ln])
                state["pairs"][k] = Ec

            def edge_phase(w):
                nt = int(T_w[w])
                if w % T_CH == 0:
                    Ttc = sT.tile([P, T_CH * G], bf16, tag="Ttc", name="Ttc")
                    nc.sync.dma_start(Ttc[:], T_d[w // T_CH])
                    state["Tchunks"] = {w // T_CH: Ttc}
                    state["Tchunks"].update(state.get("Tprev") or {})
                    state["Tprev"] = {w // T_CH: Ttc}
                if w % 2 == 0:
                    load_pair(w // 2 + 2)
                    Ec = state["pairs"].pop(w // 2)
                    state["Ec"] = Ec
                    state["Eoff"] = 0
                    state["g2s"] = sg2s.tile([P, 2 * H], bf16, tag="g2s",
                                             name="g2s")
                else:
                    Ec = state["Ec"]
                Eo = state["Eoff"]
                state["Eoff"] = Eo + nt * EW
                g2 = g2p.tile([P, H], f32, tag="g2")
                npair = nt // 2
                has_single = nt % 2 == 1
                soff = Eo + nt * H
                for pr in range(npair):
                    lhs = Ec[:, soff + 2 * pr * P : soff + (2 * pr + 2) * P]
                    lhs = lhs.rearrange("p (i m) -> p i m", i=2)
                    rhs2 = Ec[:, Eo + 2 * pr * H : Eo + (2 * pr + 2) * H]
                    rhs2 = rhs2.rearrange("p (i f) -> p i f", i=2)
                    for half in range(2):
                        lo = half * 512
                        nc.tensor.matmul(
                            g2[:, lo : lo + 512], lhs, rhs2[:, :, lo : lo + 512],
                            start=(pr == 0),
                            stop=(pr == npair - 1 and not has_single),
                            perf_mode=DR,
                        )
                if has_single:
                    t = nt - 1
                    lhs = Ec[:, soff + t * P : soff + (t + 1) * P]
                    for half in range(2):
                        lo = half * 512
                        nc.tensor.matmul(
                            g2[:, lo : lo + 512], lhs,
                            Ec[:, Eo + t * H + lo : Eo + t * H + lo + 512],
                            start=(npair == 0), stop=True,
                        )
                # evacuate (scaled) into this pair's g2s slot
                g2s = state["g2s"]
                o = (w % 2) * H
                nc.vector.tensor_scalar_mul(g2s[:, o : o + 512], g2[:, :512],
                                            evac_c)
                nc.scalar.activation(
                    g2s[:, o + 512 : o + H], g2[:, 512:],
                    mybir.ActivationFunctionType.Copy, scale=evac_c,
                )
                if w % 2 == 1 or w == NW - 1:
                    g2T = sg2T.tile([P, 16, P], bf16, tag="g2T", name="g2T")
                    nc.scalar.dma_start_transpose(
                        g2T[:, : 8 * (w % 2 + 1)], g2s[:, : H * (w % 2 + 1)]
                    )
                    g28 = sg28.tile([P, 16, P], f8, tag="g28", name="g28")
                    nc.vector.tensor_copy(g28[:, :8], g2T[:, :8])
                    if w % 2 == 1:
                        nc.scalar.activation(
                            g28[:, 8:], g2T[:, 8:],
                            mybir.ActivationFunctionType.Copy,
                        )
                    pend.append((g28, dict(state["Tchunks"]), w - (w % 2)))

            def h2_pool_phase():
                g28p, Tchunks, w0 = pend.pop(0)
                for w in (w0, w0 + 1):
                    if w >= NW:
                        continue
                    jo = 8 * (w % 2)
                    Ttc = Tchunks[w // T_CH]
                    toff = (w % T_CH) * G
                    h2p = hp.tile([P, H], f32, tag="h2p")
                    h2s = sh2.tile([P, H], bf16, tag="h2s")
                    for half in range(2):
                        lo = half * 512
                        for i in range(4):
                            nc.tensor.matmul(
                                h2p[:, lo : lo + 512],
                                g28p[:, jo + 2 * i : jo + 2 * i + 2, :],
                                W2s[:, 2 * i : 2 * i + 2, lo : lo + 512],
                                start=(i == 0), stop=False,
                                perf_mode=DR,
                            )
                        nc.tensor.matmul(
                            h2p[:, lo : lo + 512], ones1[:1, :],
                            b2s[:1, lo : lo + 512],
                            start=False, stop=True,
                        )
                        if half == 0:
                            nc.vector.tensor_scalar(
                                h2s[:, :512], h2p[:, :512], h2_c, 0.0, mult,
                                vmax,
                            )
                        else:
                            nc.scalar.activation(
                                h2s[:, 512:], h2p[:, 512:], Relu, scale=h2_c
                            )
                    for half in range(2):
                        lo = half * 512
                        nc.tensor.matmul(
                            pg3[:, lo : lo + 512],
                            Ttc[:, toff : toff + G],
                            h2s[:, lo : lo + 512],
                            start=(w == 0), stop=(w == NW - 1),
                        )

            load_pair(0)
            load_pair(1)
            for m in range((NW + 1) // 2):
                if m >= 2:
                    h2_pool_phase()
                edge_phase(2 * m)
                if 2 * m + 1 < NW:
                    edge_phase(2 * m + 1)
            h2_pool_phase()
            h2_pool_phase()

            pg3s = cst.tile([G, H], f32, tag="pg3s")
            nc.vector.tensor_copy(pg3s[:, :512], pg3[:, :512])
            nc.scalar.activation(
                pg3s[:, 512:], pg3[:, 512:], mybir.ActivationFunctionType.Copy
            )
            nc.sync.dma_start(out_d[:], pg3s[:])

    nc.finalize()
    return nc


def kernel(x, W1, b1, W2, b2, W3, b3, Wlin, blin, edge_index, batch, num_graphs):
    global LAST_RESULT
    import ml_dtypes
    from concourse.bass_utils import run_bass_kernel_spmd

    bf = ml_dtypes.bfloat16
    f8 = ml_dtypes.float8_e4m3
    x = np.asarray(x, dtype=np.float32)
    W1 = np.asarray(W1, dtype=np.float32)
    b1 = np.asarray(b1, dtype=np.float32)
    W2 = np.asarray(W2, dtype=np.float32)
    b2 = np.asarray(b2, dtype=np.float32)
    W3 = np.asarray(W3, dtype=np.float32)
    b3 = np.asarray(b3, dtype=np.float32)
    Wlin = np.asarray(Wlin, dtype=np.float32)
    blin = np.asarray(blin, dtype=np.float32)

    E, Tmat, cnt, T_w, TT, alpha_m = _host_prep(x, edge_index, batch, W1, b1)

    msg_rms_unscaled = 2.0 / alpha_m        # by construction of alpha_m
    gamma = _pow2(2.0 / (msg_rms_unscaled * 0.45))
    beta = _pow2(0.5 / float(np.sqrt((W2 ** 2).mean())))
    evac_c = gamma / (alpha_m * ALPHA_S)
    h2_c = 1.0 / (gamma * beta)

    nc = _build_device_program(TT, T_w, evac_c, h2_c)

    W2r = np.ascontiguousarray(
        (beta * W2).reshape(8, P, H).transpose(1, 0, 2).reshape(P, 8 * H)
    ).astype(f8)
    b2r = (gamma * beta * b2).reshape(1, H).astype(bf)

    in_maps = []
    for c in range(N_CORES):
        Ec = np.ascontiguousarray(E[c].reshape(P, TT * EW))
        Tc = np.ascontiguousarray(
            Tmat[c].reshape(NW // T_CH, T_CH, P, G).transpose(0, 2, 1, 3)
            .reshape(NW // T_CH, P, T_CH * G)
        ).astype(bf)
        in_maps.append({"E": Ec, "T": Tc, "W2": W2r, "b2": b2r})

    res = run_bass_kernel_spmd(nc, in_maps, core_ids=list(range(N_CORES)))
    LAST_RESULT = res
    pg3 = np.zeros((G, H), dtype=np.float64)
    for r in res.results:
        pg3 += r["pg3"].astype(np.float64)
    pg3 = pg3.astype(np.float32)

    pooled = (pg3 @ W3 + cnt[:, None] * b3[None, :]) / np.maximum(cnt, 1.0)[:, None]
    out = pooled @ Wlin + blin[None, :]
    return out.astype(np.float32)


# revision 25
# speedup vs baseline: 1.3587x; 1.0843x over previous
"""GCN (3-layer, PyG-style) forward on 8 Trainium2 NeuronCores — fp8 v3.

Math restructuring (aggregation commutes with the weight matmul; pooling is
linear):
  agg1 = Anorm @ x                        # [N,2]  tiny -> host, f64
  h1   = relu(agg1 @ W1 + b1)             # [N,1024] exact f32 on HOST
  msg  = h1[row_e]                        # gathered per edge slot, fp8, DMA'd
  g2   = Anorm @ h1                       # fp8 DoubleRow scatter matmuls
  h2   = relu(g2 @ W2 + b2)               # fp8 DoubleRow dense matmuls
  pg3  = T.T @ h2  (T[src,g]=sum norm)    # bf16, accumulated in PSUM
  out  = ((pg3 @ W3 + cnt*b3)/max(cnt,1)) @ Wlin + blin   # host, f64

Per 128-edge tile the device consumes 1152 fp8 bytes per edge slot (1024 msg
features + 128 scatter-matrix row), streamed as one contiguous per-window DMA.
Power-of-two scale factors keep every fp8 tensor in range (lossless to fold).
Nodes are assigned to the 8*98 (core,window) bins by a balanced greedy (LPT on
in-degree) so every window needs the same number of 128-edge tiles.  g2 is
transposed with the XBAR DMA-transpose; pg3 stays resident in PSUM across all
windows.  The edge phase of window w+1 is emitted before the h2 phase of
window w so the PE never idles on the g2 copy/transpose.
"""

import numpy as np

N_NODES = 100000
N_EDGES = 400000
G = 128
FIN = 2
H = 1024
N_CORES = 8
P = 128
NW = 98                       # windows per core (98*128 = 12544 >= 12500)
NSLOT = NW * P                # node slots per core
NBIN = N_CORES * NW
EW = H + P                    # fp8 bytes per edge slot (msg + S row)
T_CH = 7                      # T windows per DMA chunk (98 = 14*7)
ALPHA_S = 4.0                 # power-of-2 scale on S (norm values)

LAST_RESULT = None


def _pow2(v):
    return float(2.0 ** np.round(np.log2(v)))


def _host_prep(x, edge_index, batch, W1, b1):
    """All O(E) index work in numpy; returns per-core device arrays."""
    import heapq
    import ml_dtypes

    f8 = ml_dtypes.float8_e4m3
    x = np.asarray(x, dtype=np.float32)
    ei = np.asarray(edge_index).astype(np.int64)
    batch = np.asarray(batch).astype(np.int64)
    n = N_NODES

    loops = np.arange(n, dtype=np.int64)
    row = np.concatenate([ei[0], loops])
    col = np.concatenate([ei[1], loops])

    deg = np.bincount(col, minlength=n).astype(np.float64)
    dis = np.where(deg > 0, 1.0 / np.sqrt(np.maximum(deg, 1.0)), 0.0)
    norm = dis[row] * dis[col]

    # layer-1 aggregation (FIN=2) on host, f64
    agg1 = np.empty((n, FIN), dtype=np.float32)
    for f in range(FIN):
        agg1[:, f] = np.bincount(
            col, weights=norm * x[row, f].astype(np.float64), minlength=n
        ).astype(np.float32)
    norm = norm.astype(np.float32)

    # exact h1 per node, then fp8 with a power-of-2 scale
    h1 = np.maximum(agg1 @ W1 + b1, 0.0)
    msg_rms = float(np.sqrt((h1[:4096] ** 2).mean()))
    alpha_m = _pow2(2.0 / msg_rms)
    h1q = (alpha_m * h1).astype(f8)

    # ---- balanced node -> (core, window, pos) assignment (LPT greedy) ----
    w_node = np.bincount(col, minlength=n)          # in-degree incl self-loop
    order = np.argsort(-w_node, kind="stable")
    bin_of = np.empty(n, dtype=np.int64)
    pos_of = np.empty(n, dtype=np.int64)
    counts = np.zeros(NBIN, dtype=np.int64)
    loads = np.zeros(NBIN, dtype=np.int64)
    heap = [(0, b) for b in range(NBIN)]
    heapq.heapify(heap)
    wl = w_node[order]
    for i in range(n):
        node = order[i]
        load, b = heapq.heappop(heap)
        bin_of[node] = b
        pos_of[node] = counts[b]
        counts[b] += 1
        loads[b] = load + wl[i]
        if counts[b] < P:
            heapq.heappush(heap, (loads[b], b))
    brank = np.argsort(-loads, kind="stable")
    core_of_bin = np.empty(NBIN, dtype=np.int64)
    win_of_bin = np.empty(NBIN, dtype=np.int64)
    core_of_bin[brank] = np.arange(NBIN) % N_CORES
    win_of_bin[brank] = np.arange(NBIN) // N_CORES

    node_core = core_of_bin[bin_of]
    node_win = win_of_bin[bin_of]
    node_pos = pos_of

    # ---- edge partition by destination bin ----
    ec = node_core[col]
    ew = node_win[col]
    ekey = ec * NW + ew
    eorder = np.argsort(ekey, kind="stable")
    row_s, col_s, norm_s = row[eorder], col[eorder], norm[eorder]
    ekey_s = ekey[eorder]

    cw_counts = np.bincount(ekey_s, minlength=NBIN).reshape(N_CORES, NW)
    tiles_per_cw = (cw_counts + P - 1) // P
    T_w = tiles_per_cw.max(axis=0)
    base_tile = np.concatenate([[0], np.cumsum(T_w)])
    TT = int(base_tile[-1])

    cw_starts = np.concatenate([[0], np.cumsum(cw_counts.reshape(-1))])
    idx_in_cw = np.arange(len(col_s)) - cw_starts[ekey_s]
    tile_in_w = idx_in_cw // P
    pos = idx_in_cw % P
    tile_global = base_tile[ekey_s % NW] + tile_in_w

    # per-window payload: [nt*1024 msg features | nt*128 S rows], all fp8.
    # msg pairs are 1024 apart and S pairs 128 apart -> contiguous DoubleRow
    # access patterns for the scatter matmuls.
    c = ec_s = ec[eorder]
    Amsg = np.zeros((N_CORES, P, TT, H), dtype=f8)
    Asca = np.zeros((N_CORES, P, TT, P), dtype=f8)
    Amsg[c, pos, tile_global] = h1q[row_s]
    Asca[c, pos, tile_global, node_pos[col_s]] = (ALPHA_S * norm_s).astype(f8)
    E = np.empty((N_CORES, P, TT * EW), dtype=f8)
    for w in range(NW):
        t0, nt = int(base_tile[w]), int(T_w[w])
        woff = t0 * EW
        mend = woff + nt * H
        E[:, :, woff:mend] = Amsg[:, :, t0 : t0 + nt].reshape(N_CORES, P, nt * H)
        E[:, :, mend : mend + nt * P] = Asca[:, :, t0 : t0 + nt].reshape(
            N_CORES, P, nt * P
        )
    del Amsg, Asca

    # ---- L3 pooling matrix T: rows indexed by source-node slot ----
    gcol = batch[col]
    rowslot = node_core[row] * NSLOT + node_win[row] * P + node_pos[row]
    Tmat = np.bincount(
        rowslot * G + gcol, weights=norm.astype(np.float64),
        minlength=N_CORES * NSLOT * G,
    ).astype(np.float32).reshape(N_CORES, NW, P, G)

    cnt = np.bincount(batch, minlength=G).astype(np.float32)
    return E, Tmat, cnt, T_w, TT, alpha_m


def _build_device_program(TT, T_w, evac_c, h2_c):
    import concourse.mybir as mybir
    import concourse.tile as tile
    from concourse import bacc

    f32 = mybir.dt.float32
    bf16 = mybir.dt.bfloat16
    f8 = mybir.dt.float8e4
    nc = bacc.Bacc(None, target_bir_lowering=False, debug=False)
    DR = mybir.MatmulPerfMode.DoubleRow

    E_d = nc.dram_tensor("E", [P, TT * EW], f8, kind="ExternalInput")
    T_d = nc.dram_tensor("T", [NW // T_CH, P, T_CH * G], bf16, kind="ExternalInput")
    W2_d = nc.dram_tensor("W2", [P, 8 * H], f8, kind="ExternalInput")
    b2_d = nc.dram_tensor("b2", [1, H], bf16, kind="ExternalInput")
    out_d = nc.dram_tensor("pg3", [G, H], f32, kind="ExternalOutput")

    Relu = mybir.ActivationFunctionType.Relu
    mult = mybir.AluOpType.mult
    vmax = mybir.AluOpType.max

    with tile.TileContext(nc) as tc:
        with (
            tc.tile_pool(name="const", bufs=1) as cst,
            tc.tile_pool(name="sE", bufs=3) as sE,
            tc.tile_pool(name="sT", bufs=2) as sT,
            tc.tile_pool(name="sg2s", bufs=3) as sg2s,
            tc.tile_pool(name="sg2T", bufs=3) as sg2T,
            tc.tile_pool(name="sg28", bufs=4) as sg28,
            tc.tile_pool(name="sh2", bufs=2) as sh2,
            tc.tile_pool(name="g2p", bufs=2, space="PSUM") as g2p,
            tc.tile_pool(name="hp", bufs=1, space="PSUM") as hp,
            tc.tile_pool(name="pgp", bufs=1, space="PSUM") as pgp,
        ):
            W2s = cst.tile([P, 8, H], f8, tag="W2s")
            nc.sync.dma_start(W2s[:], W2_d[:].rearrange("p (j f) -> p j f", j=8))
            b2s = cst.tile([1, H], bf16, tag="b2s")
            nc.sync.dma_start(b2s[:], b2_d[:])
            ones1 = cst.tile([1, P], bf16, tag="ones1")
            nc.vector.memset(ones1[:], 1.0)

            pg3 = pgp.tile([G, H], f32, tag="pg3")

            state = {"Ttc": None, "pairs": {}, "Tchunks": {}}
            pend = []  # (g28 pair tile, Ttc, w0)
            base_tile = np.concatenate([[0], np.cumsum(T_w)]).astype(int)

            def load_pair(k):
                w0 = 2 * k
                if w0 >= NW:
                    return
                ln = int(T_w[w0]) * EW
                if w0 + 1 < NW:
                    ln += int(T_w[w0 + 1]) * EW
                Ec = sE.tile([P, 12 * EW], f8, tag="Ec", name="Ec")
                off = int(base_tile[w0]) * EW
                nc.sync.dma_start(Ec[:, :ln], E_d[:, off : off + ln])
                state["pairs"][k] = Ec

            def edge_phase(w):
                nt = int(T_w[w])
                if w % T_CH == 0:
                    Ttc = sT.tile([P, T_CH * G], bf16, tag="Ttc", name="Ttc")
                    nc.sync.dma_start(Ttc[:], T_d[w // T_CH])
                    state["Tchunks"] = {w // T_CH: Ttc}
                    state["Tchunks"].update(state.get("Tprev") or {})
                    state["Tprev"] = {w // T_CH: Ttc}
                if w % 2 == 0:
                    load_pair(w // 2 + 2)
                    Ec = state["pairs"].pop(w // 2)
                    state["Ec"] = Ec
                    state["Eoff"] = 0
                    state["g2s"] = sg2s.tile([P, 2 * H], bf16, tag="g2s",
                                             name="g2s")
                else:
                    Ec = state["Ec"]
                Eo = state["Eoff"]
                state["Eoff"] = Eo + nt * EW
                g2 = g2p.tile([P, H], f32, tag="g2")
                npair = nt // 2
                has_single = nt % 2 == 1
                soff = Eo + nt * H
                for pr in range(npair):
                    lhs = Ec[:, soff + 2 * pr * P : soff + (2 * pr + 2) * P]
                    lhs = lhs.rearrange("p (i m) -> p i m", i=2)
                    rhs2 = Ec[:, Eo + 2 * pr * H : Eo + (2 * pr + 2) * H]
                    rhs2 = rhs2.rearrange("p (i f) -> p i f", i=2)
                    for half in range(2):
                        lo = half * 512
                        nc.tensor.matmul(
                            g2[:, lo : lo + 512], lhs, rhs2[:, :, lo : lo + 512],
                            start=(pr == 0),
                            stop=(pr == npair - 1 and not has_single),
                            perf_mode=DR,
                        )
                if has_single:
                    t = nt - 1
                    lhs = Ec[:, soff + t * P : soff + (t + 1) * P]
                    for half in range(2):
                        lo = half * 512
                        nc.tensor.matmul(
                            g2[:, lo : lo + 512], lhs,
                            Ec[:, Eo + t * H + lo : Eo + t * H + lo + 512],
                            start=(npair == 0), stop=True,
                        )
                # evacuate (scaled) into this pair's g2s slot — ACT only, so
                # the g2 chain lives on one queue (evac -> transpose)
                g2s = state["g2s"]
                o = (w % 2) * H
                nc.scalar.activation(
                    g2s[:, o : o + H], g2[:],
                    mybir.ActivationFunctionType.Copy, scale=evac_c,
                )
                if w % 2 == 1 or w == NW - 1:
                    g2T = sg2T.tile([P, 16, P], bf16, tag="g2T", name="g2T")
                    nc.scalar.dma_start_transpose(
                        g2T[:, : 8 * (w % 2 + 1)], g2s[:, : H * (w % 2 + 1)]
                    )
                    g28 = sg28.tile([P, 16, P], f8, tag="g28", name="g28")
                    nc.gpsimd.tensor_copy(
                        g28[:, : 8 * (w % 2 + 1)], g2T[:, : 8 * (w % 2 + 1)]
                    )
                    pend.append((g28, dict(state["Tchunks"]), w - (w % 2)))

            def h2_pool_phase():
                g28p, Tchunks, w0 = pend.pop(0)
                for w in (w0, w0 + 1):
                    if w >= NW:
                        continue
                    jo = 8 * (w % 2)
                    Ttc = Tchunks[w // T_CH]
                    toff = (w % T_CH) * G
                    h2p = hp.tile([P, H], f32, tag="h2p")
                    h2s = sh2.tile([P, H], bf16, tag="h2s")
                    for half in range(2):
                        lo = half * 512
                        for i in range(4):
                            nc.tensor.matmul(
                                h2p[:, lo : lo + 512],
                                g28p[:, jo + 2 * i : jo + 2 * i + 2, :],
                                W2s[:, 2 * i : 2 * i + 2, lo : lo + 512],
                                start=(i == 0), stop=False,
                                perf_mode=DR,
                            )
                        nc.tensor.matmul(
                            h2p[:, lo : lo + 512], ones1[:1, :],
                            b2s[:1, lo : lo + 512],
                            start=False, stop=True,
                        )
                        nc.vector.tensor_scalar(
                            h2s[:, lo : lo + 512], h2p[:, lo : lo + 512],
                            h2_c, 0.0, mult, vmax,
                        )
                    for half in range(2):
                        lo = half * 512
                        nc.tensor.matmul(
                            pg3[:, lo : lo + 512],
                            Ttc[:, toff : toff + G],
                            h2s[:, lo : lo + 512],
                            start=(w == 0), stop=(w == NW - 1),
                        )

            load_pair(0)
            load_pair(1)
            for m in range((NW + 1) // 2):
                if m >= 2:
                    h2_pool_phase()
                edge_phase(2 * m)
                if 2 * m + 1 < NW:
                    edge_phase(2 * m + 1)
            h2_pool_phase()
            h2_pool_phase()

            pg3s = cst.tile([G, H], f32, tag="pg3s")
            nc.vector.tensor_copy(pg3s[:, :512], pg3[:, :512])
            nc.scalar.activation(
                pg3s[:, 512:], pg3[:, 512:], mybir.ActivationFunctionType.Copy
            )
            nc.sync.dma_start(out_d[:], pg3s[:])

    nc.finalize()
    return nc


def kernel(x, W1, b1, W2, b2, W3, b3, Wlin, blin, edge_index, batch, num_graphs):
    global LAST_RESULT
    import ml_dtypes
    from concourse.bass_utils import run_bass_kernel_spmd

    bf = ml_dtypes.bfloat16
    f8 = ml_dtypes.float8_e4m3
    x = np.asarray(x, dtype=np.float32)
    W1 = np.asarray(W1, dtype=np.float32)
    b1 = np.asarray(b1, dtype=np.float32)
    W2 = np.asarray(W2, dtype=np.float32)
    b2 = np.asarray(b2, dtype=np.float32)
    W3 = np.asarray(W3, dtype=np.float32)
    b3 = np.asarray(b3, dtype=np.float32)
    Wlin = np.asarray(Wlin, dtype=np.float32)
    blin = np.asarray(blin, dtype=np.float32)

    E, Tmat, cnt, T_w, TT, alpha_m = _host_prep(x, edge_index, batch, W1, b1)

    msg_rms_unscaled = 2.0 / alpha_m        # by construction of alpha_m
    gamma = _pow2(2.0 / (msg_rms_unscaled * 0.45))
    beta = _pow2(0.5 / float(np.sqrt((W2 ** 2).mean())))
    evac_c = gamma / (alpha_m * ALPHA_S)
    h2_c = 1.0 / (gamma * beta)

    nc = _build_device_program(TT, T_w, evac_c, h2_c)

    W2r = np.ascontiguousarray(
        (beta * W2).reshape(8, P, H).transpose(1, 0, 2).reshape(P, 8 * H)
    ).astype(f8)
    b2r = (gamma * beta * b2).reshape(1, H).astype(bf)

    in_maps = []
    for c in range(N_CORES):
        Ec = np.ascontiguousarray(E[c].reshape(P, TT * EW))
        Tc = np.ascontiguousarray(
            Tmat[c].reshape(NW // T_CH, T_CH, P, G).transpose(0, 2, 1, 3)
            .reshape(NW // T_CH, P, T_CH * G)
        ).astype(bf)
        in_maps.append({"E": Ec, "T": Tc, "W2": W2r, "b2": b2r})

    res = run_bass_kernel_spmd(nc, in_maps, core_ids=list(range(N_CORES)))
    LAST_RESULT = res
    pg3 = np.zeros((G, H), dtype=np.float64)
    for r in res.results:
        pg3 += r["pg3"].astype(np.float64)
    pg3 = pg3.astype(np.float32)

    pooled = (pg3 @ W3 + cnt[:, None] * b3[None, :]) / np.maximum(cnt, 1.0)[:, None]
    out = pooled @ Wlin + blin[None, :]
    return out.astype(np.float32)


# revision 26
# speedup vs baseline: 1.4598x; 1.0744x over previous
"""GCN (3-layer, PyG-style) forward on 8 Trainium2 NeuronCores — fp8 v3.

Math restructuring (aggregation commutes with the weight matmul; pooling is
linear):
  agg1 = Anorm @ x                        # [N,2]  tiny -> host, f64
  h1   = relu(agg1 @ W1 + b1)             # [N,1024] exact f32 on HOST
  msg  = h1[row_e]                        # gathered per edge slot, fp8, DMA'd
  g2   = Anorm @ h1                       # fp8 DoubleRow scatter matmuls
  h2   = relu(g2 @ W2 + b2)               # fp8 DoubleRow dense matmuls
  pg3  = T.T @ h2  (T[src,g]=sum norm)    # bf16, accumulated in PSUM
  out  = ((pg3 @ W3 + cnt*b3)/max(cnt,1)) @ Wlin + blin   # host, f64

Per 128-edge tile the device consumes 1152 fp8 bytes per edge slot (1024 msg
features + 128 scatter-matrix row), streamed as one contiguous per-window DMA.
Power-of-two scale factors keep every fp8 tensor in range (lossless to fold).
Nodes are assigned to the 8*98 (core,window) bins by a balanced greedy (LPT on
in-degree) so every window needs the same number of 128-edge tiles.  g2 is
transposed with the XBAR DMA-transpose; pg3 stays resident in PSUM across all
windows.  The edge phase of window w+1 is emitted before the h2 phase of
window w so the PE never idles on the g2 copy/transpose.
"""

import numpy as np

N_NODES = 100000
N_EDGES = 400000
G = 128
FIN = 2
H = 1024
N_CORES = 8
P = 128
NW = 98                       # windows per core (98*128 = 12544 >= 12500)
NSLOT = NW * P                # node slots per core
NBIN = N_CORES * NW
EW = H + P                    # fp8 bytes per edge slot (msg + S row)
T_CH = 7                      # T windows per DMA chunk (98 = 14*7)
ALPHA_S = 4.0                 # power-of-2 scale on S (norm values)

LAST_RESULT = None


def _pow2(v):
    return float(2.0 ** np.round(np.log2(v)))


def _host_prep(x, edge_index, batch, W1, b1):
    """All O(E) index work in numpy; returns per-core device arrays."""
    import heapq
    import ml_dtypes

    f8 = ml_dtypes.float8_e4m3
    x = np.asarray(x, dtype=np.float32)
    ei = np.asarray(edge_index).astype(np.int64)
    batch = np.asarray(batch).astype(np.int64)
    n = N_NODES

    loops = np.arange(n, dtype=np.int64)
    row = np.concatenate([ei[0], loops])
    col = np.concatenate([ei[1], loops])

    deg = np.bincount(col, minlength=n).astype(np.float64)
    dis = np.where(deg > 0, 1.0 / np.sqrt(np.maximum(deg, 1.0)), 0.0)
    norm = dis[row] * dis[col]

    # layer-1 aggregation (FIN=2) on host, f64
    agg1 = np.empty((n, FIN), dtype=np.float32)
    for f in range(FIN):
        agg1[:, f] = np.bincount(
            col, weights=norm * x[row, f].astype(np.float64), minlength=n
        ).astype(np.float32)
    norm = norm.astype(np.float32)

    # exact h1 per node, then fp8 with a power-of-2 scale
    h1 = np.maximum(agg1 @ W1 + b1, 0.0)
    msg_rms = float(np.sqrt((h1[:4096] ** 2).mean()))
    alpha_m = _pow2(2.0 / msg_rms)
    h1q = (alpha_m * h1).astype(f8)

    # ---- balanced node -> (core, window, pos) assignment (LPT greedy) ----
    w_node = np.bincount(col, minlength=n)          # in-degree incl self-loop
    order = np.argsort(-w_node, kind="stable")
    bin_of = np.empty(n, dtype=np.int64)
    pos_of = np.empty(n, dtype=np.int64)
    counts = np.zeros(NBIN, dtype=np.int64)
    loads = np.zeros(NBIN, dtype=np.int64)
    heap = [(0, b) for b in range(NBIN)]
    heapq.heapify(heap)
    wl = w_node[order]
    for i in range(n):
        node = order[i]
        load, b = heapq.heappop(heap)
        bin_of[node] = b
        pos_of[node] = counts[b]
        counts[b] += 1
        loads[b] = load + wl[i]
        if counts[b] < P:
            heapq.heappush(heap, (loads[b], b))
    brank = np.argsort(-loads, kind="stable")
    core_of_bin = np.empty(NBIN, dtype=np.int64)
    win_of_bin = np.empty(NBIN, dtype=np.int64)
    core_of_bin[brank] = np.arange(NBIN) % N_CORES
    win_of_bin[brank] = np.arange(NBIN) // N_CORES

    node_core = core_of_bin[bin_of]
    node_win = win_of_bin[bin_of]
    node_pos = pos_of

    # ---- edge partition by destination bin ----
    ec = node_core[col]
    ew = node_win[col]
    ekey = ec * NW + ew
    eorder = np.argsort(ekey, kind="stable")
    row_s, col_s, norm_s = row[eorder], col[eorder], norm[eorder]
    ekey_s = ekey[eorder]

    cw_counts = np.bincount(ekey_s, minlength=NBIN).reshape(N_CORES, NW)
    tiles_per_cw = (cw_counts + P - 1) // P
    T_w = tiles_per_cw.max(axis=0)
    base_tile = np.concatenate([[0], np.cumsum(T_w)])
    TT = int(base_tile[-1])

    cw_starts = np.concatenate([[0], np.cumsum(cw_counts.reshape(-1))])
    idx_in_cw = np.arange(len(col_s)) - cw_starts[ekey_s]
    tile_in_w = idx_in_cw // P
    pos = idx_in_cw % P
    tile_global = base_tile[ekey_s % NW] + tile_in_w

    # per-window payload: [nt*1024 msg features | nt*128 S rows], all fp8.
    # msg pairs are 1024 apart and S pairs 128 apart -> contiguous DoubleRow
    # access patterns for the scatter matmuls.
    c = ec_s = ec[eorder]
    Amsg = np.zeros((N_CORES, P, TT, H), dtype=f8)
    Asca = np.zeros((N_CORES, P, TT, P), dtype=f8)
    Amsg[c, pos, tile_global] = h1q[row_s]
    Asca[c, pos, tile_global, node_pos[col_s]] = (ALPHA_S * norm_s).astype(f8)
    E = np.empty((N_CORES, P, TT * EW), dtype=f8)
    for w in range(NW):
        t0, nt = int(base_tile[w]), int(T_w[w])
        woff = t0 * EW
        mend = woff + nt * H
        E[:, :, woff:mend] = Amsg[:, :, t0 : t0 + nt].reshape(N_CORES, P, nt * H)
        E[:, :, mend : mend + nt * P] = Asca[:, :, t0 : t0 + nt].reshape(
            N_CORES, P, nt * P
        )
    del Amsg, Asca

    # ---- L3 pooling matrix T: rows indexed by source-node slot ----
    gcol = batch[col]
    rowslot = node_core[row] * NSLOT + node_win[row] * P + node_pos[row]
    Tmat = np.bincount(
        rowslot * G + gcol, weights=norm.astype(np.float64),
        minlength=N_CORES * NSLOT * G,
    ).astype(np.float32).reshape(N_CORES, NW, P, G)

    cnt = np.bincount(batch, minlength=G).astype(np.float32)
    return E, Tmat, cnt, T_w, TT, alpha_m


def _build_device_program(TT, T_w, evac_c, h2_c):
    import concourse.mybir as mybir
    import concourse.tile as tile
    from concourse import bacc

    f32 = mybir.dt.float32
    bf16 = mybir.dt.bfloat16
    f8 = mybir.dt.float8e4
    nc = bacc.Bacc(None, target_bir_lowering=False, debug=False)
    DR = mybir.MatmulPerfMode.DoubleRow

    E_d = nc.dram_tensor("E", [P, TT * EW], f8, kind="ExternalInput")
    T_d = nc.dram_tensor("T", [NW // T_CH, P, T_CH * G], bf16, kind="ExternalInput")
    W2_d = nc.dram_tensor("W2", [P, 8 * H], f8, kind="ExternalInput")
    b2_d = nc.dram_tensor("b2", [1, H], bf16, kind="ExternalInput")
    out_d = nc.dram_tensor("pg3", [G, H], f32, kind="ExternalOutput")

    Relu = mybir.ActivationFunctionType.Relu
    mult = mybir.AluOpType.mult
    vmax = mybir.AluOpType.max

    with tile.TileContext(nc) as tc:
        with (
            tc.tile_pool(name="const", bufs=1) as cst,
            tc.tile_pool(name="sE", bufs=3) as sE,
            tc.tile_pool(name="sT", bufs=2) as sT,
            tc.tile_pool(name="sg2s", bufs=3) as sg2s,
            tc.tile_pool(name="sg2T", bufs=3) as sg2T,
            tc.tile_pool(name="sg28", bufs=4) as sg28,
            tc.tile_pool(name="sh2", bufs=2) as sh2,
            tc.tile_pool(name="g2p", bufs=2, space="PSUM") as g2p,
            tc.tile_pool(name="hp", bufs=1, space="PSUM") as hp,
            tc.tile_pool(name="pgp", bufs=1, space="PSUM") as pgp,
        ):
            W2s = cst.tile([P, 8, H], f8, tag="W2s")
            nc.sync.dma_start(W2s[:], W2_d[:].rearrange("p (j f) -> p j f", j=8))
            b2s = cst.tile([1, H], bf16, tag="b2s")
            nc.sync.dma_start(b2s[:], b2_d[:])
            ones1 = cst.tile([1, P], bf16, tag="ones1")
            nc.vector.memset(ones1[:], 1.0)

            pg3 = pgp.tile([G, H], f32, tag="pg3")

            state = {"Ttc": None, "pairs": {}, "Tchunks": {}}
            pend = []  # (g28 pair tile, Ttc, w0)
            base_tile = np.concatenate([[0], np.cumsum(T_w)]).astype(int)

            def load_pair(k):
                w0 = 2 * k
                if w0 >= NW:
                    return
                ln = int(T_w[w0]) * EW
                if w0 + 1 < NW:
                    ln += int(T_w[w0 + 1]) * EW
                Ec = sE.tile([P, 12 * EW], f8, tag="Ec", name="Ec")
                off = int(base_tile[w0]) * EW
                nc.sync.dma_start(Ec[:, :ln], E_d[:, off : off + ln])
                state["pairs"][k] = Ec

            def edge_phase(w):
                nt = int(T_w[w])
                if w % T_CH == 0:
                    Ttc = sT.tile([P, T_CH * G], bf16, tag="Ttc", name="Ttc")
                    nc.sync.dma_start(Ttc[:], T_d[w // T_CH])
                    state["Tchunks"] = {w // T_CH: Ttc}
                    state["Tchunks"].update(state.get("Tprev") or {})
                    state["Tprev"] = {w // T_CH: Ttc}
                if w % 2 == 0:
                    load_pair(w // 2 + 2)
                    Ec = state["pairs"].pop(w // 2)
                    state["Ec"] = Ec
                    state["Eoff"] = 0
                    state["g2s"] = sg2s.tile([P, 2 * H], bf16, tag="g2s",
                                             name="g2s")
                else:
                    Ec = state["Ec"]
                Eo = state["Eoff"]
                state["Eoff"] = Eo + nt * EW
                g2 = g2p.tile([P, H], f32, tag="g2")
                npair = nt // 2
                has_single = nt % 2 == 1
                soff = Eo + nt * H
                for pr in range(npair):
                    lhs = Ec[:, soff + 2 * pr * P : soff + (2 * pr + 2) * P]
                    lhs = lhs.rearrange("p (i m) -> p i m", i=2)
                    rhs2 = Ec[:, Eo + 2 * pr * H : Eo + (2 * pr + 2) * H]
                    rhs2 = rhs2.rearrange("p (i f) -> p i f", i=2)
                    for half in range(2):
                        lo = half * 512
                        nc.tensor.matmul(
                            g2[:, lo : lo + 512], lhs, rhs2[:, :, lo : lo + 512],
                            start=(pr == 0),
                            stop=(pr == npair - 1 and not has_single),
                            perf_mode=DR,
                        )
                if has_single:
                    t = nt - 1
                    lhs = Ec[:, soff + t * P : soff + (t + 1) * P]
                    for half in range(2):
                        lo = half * 512
                        nc.tensor.matmul(
                            g2[:, lo : lo + 512], lhs,
                            Ec[:, Eo + t * H + lo : Eo + t * H + lo + 512],
                            start=(npair == 0), stop=True,
                        )
                # evacuate (scaled) into this pair's g2s slot — ACT only, so
                # the g2 chain lives on one queue (evac -> transpose)
                g2s = state["g2s"]
                o = (w % 2) * H
                nc.scalar.activation(
                    g2s[:, o : o + H], g2[:],
                    mybir.ActivationFunctionType.Copy, scale=evac_c,
                )
                if w % 2 == 1 or w == NW - 1:
                    g2T = sg2T.tile([P, 16, P], bf16, tag="g2T", name="g2T")
                    nc.scalar.dma_start_transpose(
                        g2T[:, : 8 * (w % 2 + 1)], g2s[:, : H * (w % 2 + 1)]
                    )
                    g28 = sg28.tile([P, 16, P], f8, tag="g28", name="g28")
                    nc.scalar.activation(
                        g28[:, :8], g2T[:, :8],
                        mybir.ActivationFunctionType.Copy,
                    )
                    if w % 2 == 1:
                        nc.gpsimd.tensor_copy(g28[:, 8:], g2T[:, 8:])
                    pend.append((g28, dict(state["Tchunks"]), w - (w % 2)))

            def h2_pool_phase():
                g28p, Tchunks, w0 = pend.pop(0)
                for w in (w0, w0 + 1):
                    if w >= NW:
                        continue
                    jo = 8 * (w % 2)
                    Ttc = Tchunks[w // T_CH]
                    toff = (w % T_CH) * G
                    h2p = hp.tile([P, H], f32, tag="h2p")
                    h2s = sh2.tile([P, H], bf16, tag="h2s")
                    for half in range(2):
                        lo = half * 512
                        for i in range(4):
                            nc.tensor.matmul(
                                h2p[:, lo : lo + 512],
                                g28p[:, jo + 2 * i : jo + 2 * i + 2, :],
                                W2s[:, 2 * i : 2 * i + 2, lo : lo + 512],
                                start=(i == 0), stop=False,
                                perf_mode=DR,
                            )
                        nc.tensor.matmul(
                            h2p[:, lo : lo + 512], ones1[:1, :],
                            b2s[:1, lo : lo + 512],
                            start=False, stop=True,
                        )
                        nc.vector.tensor_scalar(
                            h2s[:, lo : lo + 512], h2p[:, lo : lo + 512],
                            h2_c, 0.0, mult, vmax,
                        )
                    for half in range(2):
                        lo = half * 512
                        nc.tensor.matmul(
                            pg3[:, lo : lo + 512],
                            Ttc[:, toff : toff + G],
                            h2s[:, lo : lo + 512],
                            start=(w == 0), stop=(w == NW - 1),
                        )

            load_pair(0)
            load_pair(1)
            for m in range((NW + 1) // 2):
                if m >= 2:
                    h2_pool_phase()
                edge_phase(2 * m)
                if 2 * m + 1 < NW:
                    edge_phase(2 * m + 1)
            h2_pool_phase()
            h2_pool_phase()

            pg3s = cst.tile([G, H], f32, tag="pg3s")
            nc.vector.tensor_copy(pg3s[:, :512], pg3[:, :512])
            nc.scalar.activation(
                pg3s[:, 512:], pg3[:, 512:], mybir.ActivationFunctionType.Copy
            )
            nc.sync.dma_start(out_d[:], pg3s[:])

    nc.finalize()
    return nc


def kernel(x, W1, b1, W2, b2, W3, b3, Wlin, blin, edge_index, batch, num_graphs):
    global LAST_RESULT
    import ml_dtypes
    from concourse.bass_utils import run_bass_kernel_spmd

    bf = ml_dtypes.bfloat16
    f8 = ml_dtypes.float8_e4m3
    x = np.asarray(x, dtype=np.float32)
    W1 = np.asarray(W1, dtype=np.float32)
    b1 = np.asarray(b1, dtype=np.float32)
    W2 = np.asarray(W2, dtype=np.float32)
    b2 = np.asarray(b2, dtype=np.float32)
    W3 = np.asarray(W3, dtype=np.float32)
    b3 = np.asarray(b3, dtype=np.float32)
    Wlin = np.asarray(Wlin, dtype=np.float32)
    blin = np.asarray(blin, dtype=np.float32)

    E, Tmat, cnt, T_w, TT, alpha_m = _host_prep(x, edge_index, batch, W1, b1)

    msg_rms_unscaled = 2.0 / alpha_m        # by construction of alpha_m
    gamma = _pow2(2.0 / (msg_rms_unscaled * 0.45))
    beta = _pow2(0.5 / float(np.sqrt((W2 ** 2).mean())))
    evac_c = gamma / (alpha_m * ALPHA_S)
    h2_c = 1.0 / (gamma * beta)

    nc = _build_device_program(TT, T_w, evac_c, h2_c)

    W2r = np.ascontiguousarray(
        (beta * W2).reshape(8, P, H).transpose(1, 0, 2).reshape(P, 8 * H)
    ).astype(f8)
    b2r = (gamma * beta * b2).reshape(1, H).astype(bf)

    in_maps = []
    for c in range(N_CORES):
        Ec = np.ascontiguousarray(E[c].reshape(P, TT * EW))
        Tc = np.ascontiguousarray(
            Tmat[c].reshape(NW // T_CH, T_CH, P, G).transpose(0, 2, 1, 3)
            .reshape(NW // T_CH, P, T_CH * G)
        ).astype(bf)
        in_maps.append({"E": Ec, "T": Tc, "W2": W2r, "b2": b2r})

    res = run_bass_kernel_spmd(nc, in_maps, core_ids=list(range(N_CORES)))
    LAST_RESULT = res
    pg3 = np.zeros((G, H), dtype=np.float64)
    for r in res.results:
        pg3 += r["pg3"].astype(np.float64)
    pg3 = pg3.astype(np.float32)

    pooled = (pg3 @ W3 + cnt[:, None] * b3[None, :]) / np.maximum(cnt, 1.0)[:, None]
    out = pooled @ Wlin + blin[None, :]
    return out.astype(np.float32)
